# revision 5
# baseline (speedup 1.0000x reference)
"""Trainium2 Bass kernel v3 for channel attention (XCA-style), 8 NeuronCores.

Data-parallel over batch B=8 (1 item/core, no collectives).

Speed tricks vs v1 baseline:
- q/k path entirely fp8-e4m3 on the PE with DoubleRow perf mode (2 k-tiles
  contracted per matmul pass, halving instruction count; measured row rate
  is 1 cyc/row regardless): pointwise conv pairs k-chunks; depthwise pairs
  taps over a pitch-64 guarded layout (every tap window is one contiguous
  512-stream; left/right column wrap errors fixed by small DVE FMAs).
  Numerically safe: fp8 noise on q/k washes out over the 4096-deep logit
  contraction (validated: l2 rel 0.0049 vs 0.0047 all-bf16).
- v depthwise folded into attn@v on the PE for chunks 1-3
  (out = sum_tap (attnT . dwv_tap) @ shifted v_pw), on the DVE for chunk 0
  (4x tensor_scalar + 2x tensor_tensor; scalar_tensor_tensor has no DVE perf
  modes). The fused matmuls land in the k-phase where the PE would idle.
- Norm squares read the depthwise PSUM directly (overlaps evacuation);
  rsqrt via DVE quake bit-hack + 2 Newton steps, so the whole kernel stays
  on the exp_and_others ACT table (no 1.28us table reloads).
- Per-half (2048-col) correction/khat/transpose pipelines to cut the
  k->attn chain latency; k/A stages software-pipelined (k0 k1 A0 k2 A1 ..).
- PSUM in [128,1024] 2-bank groups (one tag, bufs=4 = all 8 banks); fp8
  weights prescaled x16 (undone via ACT Copy scale at evacuation).
"""

import os
import sys

import numpy as np

for _p in ("/opt/trn_rl_repo", "/root/.axon_site/_ro/trn_rl_repo"):
    if os.path.isdir(_p) and _p not in sys.path:
        sys.path.insert(0, _p)

import ml_dtypes

B, C, HH, WW = 8, 512, 64, 64
HEADS, D = 8, 64
HW = HH * WW          # 4096
G = C // 128          # 4 chunks of 128 channels (2 heads each)
NBK = 512             # one PSUM bank of fp32
PP = WW + 2           # bordered pad pitch for v (66)
NG = 66 * 64 + 2      # pitch-64 guarded flat size for q/k (4226)
WSCALE = 16.0         # fp8 weight prescale (undone at pw evacuation)
QTR = 1024            # 2-bank PSUM group width

# tap index t = dy*3+dx; flat stream offset for bank nb: 1 + nb*512 + TOFF[t]
TOFF = [dy * 64 + dx - 1 for dy in range(3) for dx in range(3)]
# DoubleRow tap pairs (a, b): dim1 stride must be >= 2 (d=1 wedges the device)
DW_PAIRS = [(0, 2), (1, 4), (3, 6), (5, 8), (None, 7)]

# v chunk whose depthwise runs on DVE (explicit); others fuse into attn@v
V_DVE_CHUNKS = (0, 1, 2, 3)

_CACHE = {}


def _build():
    from contextlib import ExitStack

    import concourse.tile as tile
    from concourse import bacc, mybir
    from concourse.ap import AP

    f32 = mybir.dt.float32
    bf16 = mybir.dt.bfloat16
    f8 = mybir.dt.float8e4
    i32 = mybir.dt.int32
    AO = mybir.AluOpType
    AF = mybir.ActivationFunctionType
    PM = mybir.MatmulPerfMode

    nc = bacc.Bacc()

    x8_ext = nc.declare_dram_parameter("x8", [128, G, HW], f8, isOutput=False)
    xb_ext = nc.declare_dram_parameter("xb", [128, G, HW], bf16, isOutput=False)
    w8_ext = {t: nc.declare_dram_parameter(f"w8{t}", [128, G, C], f8,
                                           isOutput=False) for t in "qk"}
    wv_ext = nc.declare_dram_parameter("wv", [128, G, C], bf16, isOutput=False)
    wp_ext = nc.declare_dram_parameter("wp", [128, G, C], bf16, isOutput=False)
    dg_ext = {t: nc.declare_dram_parameter(f"dg{t}", [128, G, 5, 2, 128], f8,
                                           isOutput=False) for t in "qk"}
    ndw_ext = {t: nc.declare_dram_parameter(f"ndw{t}", [128, G, 9], f32,
                                            isOutput=False) for t in "qk"}
    dwv_ext = nc.declare_dram_parameter("dwv", [128, G, 9], f32, isOutput=False)
    tsc_ext = nc.declare_dram_parameter("tsc", [128, G], f32, isOutput=False)
    id_ext = nc.declare_dram_parameter("ident", [128, 128], bf16, isOutput=False)
    out_ext = nc.declare_dram_parameter("out", [C, HW], bf16, isOutput=True)

    with ExitStack() as ctx:
        tc = ctx.enter_context(tile.TileContext(nc))
        sb = ctx.enter_context(tc.tile_pool(name="sb", bufs=1))
        ps = ctx.enter_context(tc.tile_pool(name="ps", bufs=1, space="PSUM"))

        def win(tile_, off, dims):
            a = tile_[:, off:off + 1]
            return AP(a.tensor, a.offset, [list(a.ap[0])] + dims)

        # ---- PE warm-up: ramp the pstate during the initial DMA wait ----
        warm = sb.tile([128, 128], bf16, name="warm", tag="warm")
        nc.vector.memset(warm, 0.0)
        wp_ps = ps.tile([128, NBK], f32, name="warmps", tag="ps2", bufs=4)
        for i in range(24):
            nc.tensor.matmul(wp_ps[:, 0:128], lhsT=warm, rhs=warm,
                             start=(i == 0), stop=(i == 23))

        # ---- persistent loads (q-path first so pw can start ASAP) -------
        w8q = sb.tile([128, G, C], f8, name="w8q", tag="w8q")
        nc.sync.dma_start(out=w8q, in_=w8_ext["q"][:, :, :])
        x8 = sb.tile([128, G, HW], f8, name="x8", tag="x8")
        nc.sync.dma_start(out=x8[:, 0:2, :], in_=x8_ext[:, 0:2, :])
        nc.sync.dma_start(out=x8[:, 2:4, :], in_=x8_ext[:, 2:4, :])
        dgq = sb.tile([128, G, 5, 2, 128], f8, name="dgq", tag="dgq")
        nc.sync.dma_start(out=dgq, in_=dg_ext["q"][:, :, :, :, :])
        ndwq = sb.tile([128, G, 9], f32, name="ndwq", tag="ndwq")
        nc.sync.dma_start(out=ndwq, in_=ndw_ext["q"][:, :, :])
        tsc = sb.tile([128, G], f32, name="tsc", tag="tsc")
        nc.sync.dma_start(out=tsc, in_=tsc_ext[:, :])
        w8k = sb.tile([128, G, C], f8, name="w8k", tag="w8k")
        nc.sync.dma_start(out=w8k, in_=w8_ext["k"][:, :, :])
        dgk = sb.tile([128, G, 5, 2, 128], f8, name="dgk", tag="dgk")
        nc.sync.dma_start(out=dgk, in_=dg_ext["k"][:, :, :, :, :])
        ndwk = sb.tile([128, G, 9], f32, name="ndwk", tag="ndwk")
        nc.sync.dma_start(out=ndwk, in_=ndw_ext["k"][:, :, :])
        xb = sb.tile([128, G, HW], bf16, name="xb", tag="xb")
        nc.sync.dma_start(out=xb, in_=xb_ext[:, :, :])
        wv = sb.tile([128, G, C], bf16, name="wv", tag="wvp")
        nc.sync.dma_start(out=wv, in_=wv_ext[:, :, :])
        dwv = sb.tile([128, G, 9], f32, name="dwv", tag="dwv")
        nc.sync.dma_start(out=dwv, in_=dwv_ext[:, :, :])
        ident = sb.tile([128, 128], bf16, name="ident", tag="ident")
        nc.sync.dma_start(out=ident, in_=id_ext[:, :])

        w8 = {"q": w8q, "k": w8k}
        dg = {"q": dgq, "k": dgk}
        ndw = {"q": ndwq, "k": ndwk}

        # persistent per-chunk results
        vpad = [None] * G
        qT = [sb.tile([128, HW], bf16, name=f"qT{g}", tag=f"qT{g}")
              for g in range(G)]
        ts_scale = [sb.tile([128, 1], f32, name=f"tss{g}", tag=f"tss{g}")
                    for g in range(G)]
        kT = [None] * G     # rotating pool "ktv" shared with vdw
        vdw = {}
        o_sb = [None] * G
        aexp_t = []
        for g in range(G):
            ax = sb.tile([128, 128], bf16, name=f"aexp{g}", tag=f"aexp{g}")
            nc.vector.memset(ax, 0.0)
            aexp_t.append(ax)

        # ---- q/k pointwise + depthwise, all fp8 DoubleRow ----------------
        def qk_chunk(t, g):
            """fp8 pw into guarded qg, fp8-DR depthwise into bf16 acc,
            per-quarter PSUM norm squares. Returns (acc, rinv, qg)."""
            qg = sb.tile([128, NG], f8, name=f"qg_{t}{g}", tag="qg", bufs=2)
            # zero guards: front elem + row0 [0..64], row65 + tail [4161..]
            nc.gpsimd.memset(qg[:, 0:65], 0.0)
            nc.gpsimd.memset(qg[:, 1 + 64 * 65:NG], 0.0)
            for q4 in range(4):                     # 2-bank quarters
                pw = ps.tile([128, QTR], f32, name=f"pw_{t}{g}{q4}",
                             tag="ps2", bufs=4)
                for j in range(2):                  # k-chunk pairs
                    lhsT = w8[t][:, 2 * j:2 * j + 2, g * 128:(g + 1) * 128]
                    for nb in range(2):
                        nc.tensor.matmul(
                            pw[:, nb * NBK:(nb + 1) * NBK], lhsT=lhsT,
                            rhs=x8[:, 2 * j:2 * j + 2,
                                   (2 * q4 + nb) * NBK:(2 * q4 + nb + 1) * NBK],
                            start=(j == 0), stop=(j == 1),
                            perf_mode=PM.DoubleRow)
                # rows 16*q4+1 .. 16*q4+16 of the guarded tile, contiguous
                nc.scalar.activation(
                    out=qg[:, 1 + 64 * (16 * q4 + 1):1 + 64 * (16 * q4 + 17)],
                    in_=pw, func=AF.Copy, scale=1.0 / WSCALE)
            acc = sb.tile([128, HW], bf16, name=f"acc_{t}{g}", tag="acc",
                          bufs=2)
            ssp = sb.tile([128, 4], f32, name=f"ssp_{t}{g}", tag="nrm_ssp",
                          bufs=2)
            for q4 in range(4):
                dwp = ps.tile([128, QTR], f32, name=f"dw_{t}{g}{q4}",
                              tag="ps2", bufs=4)
                for i, (ta, tb) in enumerate(DW_PAIRS):
                    offa = TOFF[ta] if ta is not None else TOFF[tb] - 2
                    d = TOFF[tb] - offa
                    lhsT = dg[t][:, g, i, :, :]
                    for nb in range(2):
                        base = 1 + (2 * q4 + nb) * NBK + offa
                        nc.tensor.matmul(
                            dwp[:, nb * NBK:(nb + 1) * NBK], lhsT=lhsT,
                            rhs=win(qg, base, [[d, 2], [1, NBK]]),
                            start=(i == 0), stop=(i == len(DW_PAIRS) - 1),
                            perf_mode=PM.DoubleRow)
                nc.scalar.copy(acc[:, q4 * QTR:(q4 + 1) * QTR], dwp)
                if t == "k":
                    # k: partial square in place on PSUM - rinv gates khat,
                    # so start the norm as early as possible
                    nc.scalar.activation(out=dwp, in_=dwp, func=AF.Square,
                                         accum_out=ssp[:, q4:q4 + 1])
            ss = sb.tile([128, 1], f32, name=f"ss_{t}{g}", tag="nrm_ss",
                         bufs=2)
            if t == "k":
                nc.vector.tensor_tensor(out=ss, in0=ssp[:, 0:1],
                                        in1=ssp[:, 1:2], op=AO.add)
                nc.vector.tensor_tensor(out=ss, in0=ss, in1=ssp[:, 2:3],
                                        op=AO.add)
                nc.vector.tensor_tensor(out=ss, in0=ss, in1=ssp[:, 3:4],
                                        op=AO.add)
            else:
                # q: rinv only feeds the exp scale much later - one SBUF
                # square keeps the PSUM groups free for the PE pipeline
                sqs = sb.tile([128, HW], bf16, name=f"sqs_{t}{g}",
                              tag="trans", bufs=1)
                nc.scalar.activation(out=sqs, in_=acc, func=AF.Square,
                                     accum_out=ss)
            sh = sb.tile([128, 1], i32, name=f"sh_{t}{g}", tag="nrm_sh",
                         bufs=2)
            nc.vector.tensor_scalar(out=sh, in0=ss.bitcast(i32), scalar1=1,
                                    scalar2=None, op0=AO.logical_shift_right)
            y0i = sb.tile([128, 1], i32, name=f"y0_{t}{g}", tag="nrm_y0",
                          bufs=2)
            eng = nc.vector
            eng.add_instruction(mybir.InstTensorScalarPtr(
                name=nc.get_next_instruction_name(),
                op0=AO.subtract, reverse0=True,
                ins=[eng.lower_ap(sh[:, :]),
                     mybir.ImmediateValue(dtype=i32, value=0x5f3759df)],
                outs=[eng.lower_ap(y0i[:, :])]))
            rinv = sb.tile([128, 1], f32, name=f"ri_{t}{g}", tag=f"ri_{t}{g}")
            nc.vector.tensor_copy(rinv, y0i.bitcast(f32))
            tn = sb.tile([128, 1], f32, name=f"tn_{t}{g}", tag="nrm_tn",
                         bufs=2)
            for _ in range(2):
                nc.vector.tensor_tensor(out=tn, in0=rinv, in1=rinv, op=AO.mult)
                nc.vector.tensor_tensor(out=tn, in0=tn, in1=ss, op=AO.mult)
                nc.vector.tensor_scalar(out=tn, in0=tn, scalar1=-0.5,
                                        scalar2=1.5, op0=AO.mult, op1=AO.add)
                nc.vector.tensor_tensor(out=rinv, in0=rinv, in1=tn, op=AO.mult)
            return acc, rinv, qg

        def corrections(t, g, acc, qg, h):
            """Edge-wrap fixes on a 2048-col half: col0/col63, negated taps."""
            acc3 = acc.rearrange("p (h w) -> p h w", w=WW)
            r0 = 32 * h
            for dy in range(3):
                nc.vector.scalar_tensor_tensor(
                    out=acc3[:, r0:r0 + 32, 0:1],
                    in0=win(qg, 64 * (r0 + dy), [[64, 32], [1, 1]]),
                    scalar=ndw[t][:, g, 3 * dy:3 * dy + 1],
                    in1=acc3[:, r0:r0 + 32, 0:1], op0=AO.mult, op1=AO.add)
                nc.vector.scalar_tensor_tensor(
                    out=acc3[:, r0:r0 + 32, 63:64],
                    in0=win(qg, 1 + 64 * (r0 + dy + 1), [[64, 32], [1, 1]]),
                    scalar=ndw[t][:, g, 3 * dy + 2:3 * dy + 3],
                    in1=acc3[:, r0:r0 + 32, 63:64], op0=AO.mult, op1=AO.add)

        def q_chunk(g):
            acc, rinv, qg = qk_chunk("q", g)
            dst3 = qT[g].rearrange("p (a c) -> p a c", c=128)
            for h in range(2):
                corrections("q", g, acc, qg, h)
                nc.sync.dma_start(out=dst3[:, 16 * h:16 * (h + 1), :],
                                  in_=acc[:, 2048 * h:2048 * (h + 1)],
                                  transpose=True)
            nc.vector.tensor_tensor(out=ts_scale[g], in0=tsc[:, g:g + 1],
                                    in1=rinv, op=AO.mult)

        def k_chunk(g):
            acc, rinv, qg = qk_chunk("k", g)
            kh = sb.tile([128, HW], bf16, name=f"kh{g}", tag="trans", bufs=1)
            kt = sb.tile([128, HW], bf16, name=f"kT{g}", tag="ktv", bufs=4)
            dst3 = kt.rearrange("p (a c) -> p a c", c=128)
            for h in range(2):
                corrections("k", g, acc, qg, h)
                nc.vector.tensor_scalar(out=kh[:, 2048 * h:2048 * (h + 1)],
                                        in0=acc[:, 2048 * h:2048 * (h + 1)],
                                        scalar1=rinv, scalar2=None,
                                        op0=AO.mult)
                nc.sync.dma_start(out=dst3[:, 16 * h:16 * (h + 1), :],
                                  in_=kh[:, 2048 * h:2048 * (h + 1)],
                                  transpose=True)
            kT[g] = kt

        # ---- v pointwise (bf16) into bordered 66-pitch pad ---------------
        def v_pw(g):
            pad = sb.tile([128, PP, PP], bf16, name=f"vpad{g}", tag="vpadp",
                          bufs=3)
            vpad[g] = pad
            padf = pad.rearrange("p h w -> p (h w)")
            nc.gpsimd.memset(padf[:, 0:PP], 0.0)
            nc.gpsimd.memset(padf[:, (PP - 1) * PP:PP * PP], 0.0)
            nc.gpsimd.memset(pad[:, 1:PP - 1, 0:1], 0.0)
            nc.gpsimd.memset(pad[:, 1:PP - 1, PP - 1:PP], 0.0)
            for q4 in range(4):
                pw = ps.tile([128, QTR], f32, name=f"vpw{g}{q4}",
                             tag="ps2", bufs=4)
                for k in range(G):
                    lhsT = wv[:, k, g * 128:(g + 1) * 128]
                    for nb in range(2):
                        nc.tensor.matmul(
                            pw[:, nb * NBK:(nb + 1) * NBK], lhsT=lhsT,
                            rhs=xb[:, k,
                                   (2 * q4 + nb) * NBK:(2 * q4 + nb + 1) * NBK],
                            start=(k == 0), stop=(k == G - 1))
                nc.scalar.copy(
                    pad[:, 1 + q4 * 16:1 + (q4 + 1) * 16, 1:WW + 1],
                    pw.rearrange("p (h w) -> p h w", w=WW))

        # ---- explicit v depthwise on DVE (4x mult + 2x add chain),
        # split into tap-slices so it never head-of-line-blocks the DVE ----
        def v_dw_dve(g, taps, seed=4):
            pad = vpad[g]
            if g not in vdw:
                acc = sb.tile([128, HW], bf16, name=f"vdw{g}", tag="ktv",
                              bufs=4)
                vdw[g] = acc
                acc3 = acc.rearrange("p (h w) -> p h w", w=WW)
                sy, sx = seed // 3, seed % 3
                nc.vector.tensor_scalar(
                    out=acc3[:, :, :], in0=pad[:, sy:sy + HH, sx:sx + WW],
                    scalar1=dwv[:, g, seed:seed + 1], scalar2=None,
                    op0=AO.mult)
            acc3 = vdw[g].rearrange("p (h w) -> p h w", w=WW)
            tmp = sb.tile([128, HH, WW], bf16, name=f"vt{g}", tag="vtmp",
                          bufs=1)
            for tap in taps:
                dy, dx = tap // 3, tap % 3
                nc.vector.tensor_scalar(
                    out=tmp, in0=pad[:, dy:dy + HH, dx:dx + WW],
                    scalar1=dwv[:, g, tap:tap + 1], scalar2=None, op0=AO.mult)
                nc.vector.tensor_tensor(out=acc3, in0=acc3, in1=tmp,
                                        op=AO.add)

        def v_dw2_gps(g, tap):
            # tap contribution in halves: DVE scaled-mult, gpsimd add
            pad = vpad[g]
            acc = vdw[g]
            dy, dx = tap // 3, tap % 3
            for h in range(4):
                tmp = sb.tile([128, 16, WW], bf16, name=f"v2t{tap}{h}",
                              tag="vtmp2", bufs=2)
                nc.vector.tensor_scalar(
                    out=tmp, in0=pad[:, dy + 16 * h:dy + 16 * (h + 1),
                                     dx:dx + WW],
                    scalar1=dwv[:, g, tap:tap + 1], scalar2=None, op0=AO.mult)
                a3v = vdw[g].rearrange("p (h w) -> p h w", w=WW)
                nc.gpsimd.tensor_tensor(
                    out=a3v[:, 16 * h:16 * (h + 1), :],
                    in0=a3v[:, 16 * h:16 * (h + 1), :], in1=tmp, op=AO.add)

        # ---- attention for one chunk (2 heads) ---------------------------
        def attn_chunk(g):
            ap_ = ps.tile([128, QTR], f32, name=f"attn{g}", tag="ps2", bufs=4)
            for nck in range(32):
                nc.tensor.matmul(
                    ap_[:, 0:128],
                    lhsT=qT[g][:, nck * 128:(nck + 1) * 128],
                    rhs=kT[g][:, nck * 128:(nck + 1) * 128],
                    start=(nck == 0), stop=(nck == 31))
            aexp = aexp_t[g]
            sume = sb.tile([128, 1], f32, name=f"sume{g}", tag="sume", bufs=2)
            for blk in (0, 64):
                nc.scalar.activation(
                    out=aexp[blk:blk + 64, blk:blk + 64],
                    in_=ap_[blk:blk + 64, blk:blk + 64],
                    func=AF.Exp, scale=ts_scale[g][blk:blk + 64, :],
                    accum_out=sume[blk:blk + 64, :])
            rs = sb.tile([128, 1], f32, name=f"rs{g}", tag="rsum", bufs=2)
            nc.vector.reciprocal(rs, sume)
            atp = ap_[:, 256:384]
            nc.tensor.matmul(atp, lhsT=aexp, rhs=ident,
                             start=True, stop=True)
            og = xb[:, g, :]          # xb slot g is dead after v_pw(3)
            if g in V_DVE_CHUNKS:
                attnT = sb.tile([128, 128], bf16, name=f"attnT{g}",
                                tag="attnT", bufs=1)
                nc.vector.tensor_copy(attnT, atp)
                for q4 in range(4):
                    vo = ps.tile([128, QTR], f32, name=f"vo{g}{q4}",
                                 tag="ps2", bufs=4)
                    for nb in range(2):
                        nc.tensor.matmul(
                            vo[:, nb * NBK:(nb + 1) * NBK], lhsT=attnT,
                            rhs=vdw[g][:, (2 * q4 + nb) * NBK:
                                       (2 * q4 + nb + 1) * NBK],
                            start=True, stop=True)
                    nc.scalar.activation(out=og[:, q4 * QTR:(q4 + 1) * QTR],
                                         in_=vo, func=AF.Copy, scale=rs)
            else:
                # fused: out = sum_tap (attnT . dwv_tap) @ shifted v_pw
                a3 = []
                for tap in range(9):
                    a3t = sb.tile([128, 128], bf16, name=f"a3_{g}{tap}",
                                  tag=f"a3_{tap}", bufs=1)
                    nc.vector.tensor_scalar(out=a3t, in0=atp,
                                            scalar1=dwv[:, g, tap:tap + 1],
                                            scalar2=None, op0=AO.mult)
                    a3.append(a3t)
                for q4 in range(4):
                    vo = ps.tile([128, QTR], f32, name=f"fo{g}{q4}",
                                 tag="ps2", bufs=4)
                    for tap in range(9):
                        dy, dx = tap // 3, tap % 3
                        for nb in range(2):
                            r0 = (2 * q4 + nb) * 8
                            nc.tensor.matmul(
                                vo[:, nb * NBK:(nb + 1) * NBK], lhsT=a3[tap],
                                rhs=vpad[g][:, dy + r0:dy + r0 + 8,
                                            dx:dx + WW],
                                start=(tap == 0), stop=(tap == 8))
                    nc.scalar.activation(out=og[:, q4 * QTR:(q4 + 1) * QTR],
                                         in_=vo, func=AF.Copy, scale=rs)
            o_sb[g] = og

        # ================= main schedule =================================
        q_chunk(0)
        q_chunk(1)
        v_pw(0)
        v_pw(1)
        v_dw_dve(0, (0, 1, 2, 3, 5))
        q_chunk(2)
        v_dw_dve(0, (6, 7, 8))
        v_dw_dve(1, (0, 1, 2))
        q_chunk(3)
        v_dw_dve(1, (3, 5, 6))
        v_pw(2)
        v_pw(3)
        v_dw_dve(1, (7, 8))
        # wv is dead now: load proj weights into its slot
        wp = sb.tile([128, G, C], bf16, name="wp", tag="wvp")
        nc.sync.dma_start(out=wp, in_=wp_ext[:, :, :])
        k_chunk(0)
        v_dw_dve(3, (0, 1))
        k_chunk(1)
        v_dw_dve(3, (2, 3))
        attn_chunk(0)
        v_dw_dve(2, (1, 2), seed=0)
        k_chunk(2)
        v_dw_dve(2, (3, 4))
        v_dw_dve(3, (5, 6))
        v_dw2_gps(2, 5)
        attn_chunk(1)
        v_dw2_gps(2, 6)
        v_dw_dve(3, (7, 8))
        v_dw2_gps(2, 7)
        k_chunk(3)
        v_dw2_gps(2, 8)
        attn_chunk(2)
        attn_chunk(3)

        # ================= projection + store ============================
        for m in range(G):
            for q4 in range(4):
                yp = ps.tile([128, QTR], f32, name=f"yp{m}{q4}",
                             tag="ps2", bufs=4)
                for g in range(G):
                    lhsT = wp[:, g, m * 128:(m + 1) * 128]
                    for nb in range(2):
                        nc.tensor.matmul(
                            yp[:, nb * NBK:(nb + 1) * NBK], lhsT=lhsT,
                            rhs=o_sb[g][:, (2 * q4 + nb) * NBK:
                                        (2 * q4 + nb + 1) * NBK],
                            start=(g == 0), stop=(g == G - 1))
                for half in range(2):
                    yt = sb.tile([128, NBK], bf16, name=f"yt{m}{q4}{half}",
                                 tag="ysb", bufs=4)
                    nc.scalar.copy(yt, yp[:, half * NBK:(half + 1) * NBK])
                    nc.sync.dma_start(
                        out=out_ext[m * 128:(m + 1) * 128,
                                    q4 * QTR + half * NBK:
                                    q4 * QTR + (half + 1) * NBK],
                        in_=yt)

    nc.compile()
    return nc


def prep_inputs(x, w_q, w_k, w_v, dw_q, dw_k, dw_v, w_proj, temperature):
    bf = ml_dtypes.bfloat16
    f8 = ml_dtypes.float8_e4m3
    xf = np.ascontiguousarray(np.asarray(x, np.float32)).reshape(B, C, HW)
    # [C, HW] -> [128, G, HW]
    xg = xf.reshape(B, G, 128, HW).transpose(0, 2, 1, 3)

    def wprep(w, dtype, scale=1.0):
        # w [O, I] -> lhsT layout [128 (i in chunk), G (i chunk), O]
        wt = (np.asarray(w, np.float32).T * scale).reshape(G, 128, C)
        return np.ascontiguousarray(wt.transpose(1, 0, 2)).astype(dtype)

    def dwprep(dw):
        # [C,1,3,3] -> [128, G, 9]
        d = np.asarray(dw, np.float32).reshape(G, 128, 9)
        return np.ascontiguousarray(d.transpose(1, 0, 2))

    def dgprep(dw9):
        # dw9 [128, G, 9] f32 (already fp8-rounded) -> diag pairs
        dgt = np.zeros((128, G, 5, 2, 128), np.float32)
        r = np.arange(128)
        for i, (ta, tb) in enumerate(DW_PAIRS):
            if ta is not None:
                dgt[r, :, i, 0, r] = dw9[r, :, ta]
            dgt[r, :, i, 1, r] = dw9[r, :, tb]
        return dgt.astype(f8)

    dwq9 = dwprep(dw_q).astype(f8).astype(np.float32)
    dwk9 = dwprep(dw_k).astype(f8).astype(np.float32)
    base = {
        "w8q": wprep(w_q, f8, WSCALE),
        "w8k": wprep(w_k, f8, WSCALE),
        "wv": wprep(w_v, bf),
        "wp": wprep(w_proj, bf),
        "dgq": dgprep(dwq9),
        "dgk": dgprep(dwk9),
        "ndwq": np.ascontiguousarray(-dwq9),
        "ndwk": np.ascontiguousarray(-dwk9),
        "dwv": dwprep(dw_v),
        "tsc": np.ascontiguousarray(np.repeat(
            np.asarray(temperature, np.float32).reshape(HEADS), D)
            .reshape(G, 128).T),
        "ident": np.eye(128, dtype=bf),
    }
    in_maps = []
    for b in range(B):
        m = dict(base)
        m["x8"] = np.ascontiguousarray(xg[b]).astype(f8)
        m["xb"] = np.ascontiguousarray(xg[b]).astype(bf)
        in_maps.append(m)
    return in_maps


def run(trace=False, **inputs):
    from concourse.bass_utils import run_bass_kernel_spmd

    if "nc" not in _CACHE:
        _CACHE["nc"] = _build()
    nc = _CACHE["nc"]
    in_maps = prep_inputs(**inputs)
    res = run_bass_kernel_spmd(nc, in_maps, core_ids=list(range(B)),
                               trace=trace)
    out = np.stack([np.asarray(res.results[b]["out"], np.float32)
                    for b in range(B)])
    return out.reshape(B, C, HH, WW), res


def kernel(**inputs):
    out, _ = run(trace=False, **inputs)
    return out
